# revision 4
# baseline (speedup 1.0000x reference)
"""BinaryTreeCRF inside-algorithm kernel for 8 Trainium2 NeuronCores.

Strategy (hardcoded for hidden=[16383,1024], L=32, depth 13):
  - The 16383-node heap tree is cut at big-tree level 3: each of the 8 cores
    owns the 2047-node subtree rooted at heap node 7+c (big levels 3..13).
  - Per core, node hidden states ship as fp8_e4m3 (half the HBM bytes of
    bf16; quantization error is ~1e-4 relative against the 2e-2 gate),
    pre-transposed to [128, chunk, col] layout with tree levels bit-reversed
    so left/right children are contiguous half-blocks, and columns grouped
    into pass-major blocks so each combine pass depends on one HBM block.
  - On device:
      E^T = (32W)^T fp8 @ hs fp8 via DoubleRow perf-mode (2 K-chunks per
      pass through the PE), then E = psE/32 + b on the DVE.
      Three combine levels (1024 leaves -> 512 -> 256 nodes) use the
      exp-factorized contraction (no [L^2, nj] logP tensor, no mean
      subtraction -- f32/bf16 exponent range covers resid <= ~30):
        Pl = exp(rl), Pr = exp(rr)                    (ACT, [L, nj])
        U[(k l), j] = sum_r T2[(k l), r] Pr[r, j]      (PE, 8 chunks)
        V = U * rep4(Pl)                               (DVE, PSUM read)
        S[k, j] = sum_l V[(k l), j]                    (PE, selector accum)
        resid' = elev + ln S                           (ACT + DVE)
  - Host finishes levels 3..10 per core + big-tree top 3 levels in float64
    (~9% of FLOPs, no HBM traffic: only E-tail [32,256] + resid2 ship back).
"""

import numpy as np
import ml_dtypes

BF16 = ml_dtypes.bfloat16
FP8 = ml_dtypes.float8_e4m3   # TRN fp8_exp4 (max normal 240)

INPUT_SIZE = 1024
L = 32
DEPTH = 13
N_CORES = 8
SUB_LEVELS = 11       # per-core subtree levels: 0 = 1024 leaves ... 10 = root
COLS = 2048           # per-core columns (2047 nodes + 1 zero pad)
WSCALE = 32.0         # W is scaled by 32 before e4m3 cast (avoids subnormals)

# "old" layout: levels from the leaves up, each level bit-reversed.
OFFS = []
_o = 0
for _l in range(SUB_LEVELS):
    OFFS.append(_o)
    _o += 1 << (10 - _l)
assert _o == 2047

# "new" (pass-major) layout: two 768-col blocks [rl | rr | elev] for the two
# level-1 passes (256 parents each), then the level-2 elev block + host tail.
BLOCK_SIZES = [768, 768, 256, 256]
BLOCK_STARTS = np.concatenate([[0], np.cumsum(BLOCK_SIZES)])[:-1]
NEWCOL_TO_OLD = np.empty(COLS, dtype=np.int64)
for _g in range(2):
    _b = _g * 768
    NEWCOL_TO_OLD[_b:_b + 256] = np.arange(_g * 256, _g * 256 + 256)
    NEWCOL_TO_OLD[_b + 256:_b + 512] = 512 + np.arange(_g * 256, _g * 256 + 256)
    NEWCOL_TO_OLD[_b + 512:_b + 768] = 1024 + np.arange(_g * 256, _g * 256 + 256)
NEWCOL_TO_OLD[1536:COLS] = np.arange(1536, COLS)


def _bitrev(x, bits):
    x = np.asarray(x, dtype=np.int64)
    out = np.zeros_like(x)
    for i in range(bits):
        out = (out << 1) | ((x >> i) & 1)
    return out


def _core_col_heap_index(c):
    """heap index for each of the 2047 real old-layout columns of core c."""
    idx = np.zeros(2047, dtype=np.int64)
    for lev in range(SUB_LEVELS):
        m = 1 << (10 - lev)
        d = DEPTH - lev
        q = np.arange(m)
        j = _bitrev(q, 10 - lev)
        idx[OFFS[lev]: OFFS[lev] + m] = (1 << d) - 1 + c * m + j
    return idx


_NC = None


def _build_bass():
    global _NC
    if _NC is not None:
        return _NC
    from concourse import bacc, mybir
    from concourse.tile import TileContext

    dt8 = mybir.dt.float8e4
    dtb = mybir.dt.bfloat16
    dtf = mybir.dt.float32
    AF = mybir.ActivationFunctionType
    MUL = mybir.AluOpType.mult
    ADD = mybir.AluOpType.add
    DR = mybir.MatmulPerfMode.DoubleRow

    nc = bacc.Bacc()
    hsB = [nc.dram_tensor(f"hsB{g}", [128, 8 * BLOCK_SIZES[g]], dt8,
                          kind="ExternalInput") for g in range(4)]
    cWd = nc.dram_tensor("cW", [128, 256], dt8, kind="ExternalInput")
    c128d = nc.dram_tensor("c128", [128, 256], dtb, kind="ExternalInput")
    c32d = nc.dram_tensor("c32", [L, 1153], dtb, kind="ExternalInput")
    outE = nc.dram_tensor("outE", [L, 256], dtb, kind="ExternalOutput")
    outResid = nc.dram_tensor("outResid", [L, 256], dtf, kind="ExternalOutput")

    with TileContext(nc) as tc:
        with tc.tile_pool(name="consts", bufs=1) as consts, \
             tc.tile_pool(name="hs", bufs=1) as hpool, \
             tc.tile_pool(name="state", bufs=1) as state, \
             tc.tile_pool(name="vbuf", bufs=2) as vbuf, \
             tc.tile_pool(name="tmp", bufs=4) as tmp, \
             tc.tile_pool(name="pse", bufs=1, space="PSUM") as pse, \
             tc.tile_pool(name="psu", bufs=4, space="PSUM") as psu, \
             tc.tile_pool(name="pss", bufs=1, space="PSUM") as pss:

            # ---- input DMAs, all on the sync HWDGE ring: strict FIFO so
            # blocks land in chase order at full HBM bandwidth.
            c32_t = consts.tile([L, 1153], dtb, tag="c32")
            nc.sync.dma_start(out=c32_t, in_=c32d[:, :])
            t2T_t = c32_t[:, 0:1024]        # [32, 8*128] texp chunks
            rep4_t = c32_t[:, 1024:1152]    # [32, 128] partition-replicate
            bias_t = c32_t[:, 1152:1153]    # [32, 1] bf16 bias

            cW_t = consts.tile([128, 8, 32], dt8, tag="cW")
            nc.sync.dma_start(out=cW_t, in_=cWd[:, :].rearrange(
                "p (c m) -> p c m", c=8))

            hs_t = []
            for g in range(4):
                n = BLOCK_SIZES[g]
                hs_g = hpool.tile([128, 8, n], dt8, tag=f"hs{g}")
                hs_t.append(hs_g)

            def load_block(g, pieces):
                n = BLOCK_SIZES[g]
                for q in range(pieces):
                    w = 8 // pieces
                    nc.sync.dma_start(
                        out=hs_t[g][:, q * w:(q + 1) * w, :],
                        in_=hsB[g][:, q * w * n:(q + 1) * w * n].rearrange(
                            "p (c n) -> p c n", c=w))

            load_block(0, 4)
            sel8_t = consts.tile([128, 8, 32], dtb, tag="sel8")
            nc.sync.dma_start(out=sel8_t, in_=c128d[:, :].rearrange(
                "p (c m) -> p c m", c=8))
            load_block(1, 4)
            load_block(2, 1)
            load_block(3, 1)

            # Upcast bias to f32 (tensor_scalar needs an f32 scalar AP);
            # doubles as an ACT warm-up that absorbs the const-DMA wait.
            bias_f = tmp.tile([L, 1], dtf, tag="bias_f")
            nc.scalar.activation(out=bias_f, in_=bias_t, func=AF.Identity)

            # PE warm-up: junk matmuls on the const tile keep the PE HAM busy
            # through the load phase so real matmuls run at 2.4 GHz. They
            # share the psE512 ring slot, which E0 would wait on anyway.
            junk = pse.tile([L, 512], dtf, tag="psE512")
            for _ in range(5):
                nc.tensor.matmul(junk, lhsT=t2T_t[:, 0:32],
                                 rhs=t2T_t[:, 0:512], start=True, stop=True)

            E_bf = state.tile([L, COLS], dtb, tag="E_bf")
            resid1 = state.tile([L, 512], dtb, tag="resid1")
            resid2 = state.tile([L, 256], dtf, tag="resid2")

            def emit_E(g):
                s = int(BLOCK_STARTS[g])
                n = BLOCK_SIZES[g]
                nbs = [(0, min(n, 512))] + ([(512, n - 512)] if n > 512 else [])
                for o, w in nbs:
                    psE = pse.tile([L, w], dtf, tag=f"psE{w}")
                    for p in range(4):
                        nc.tensor.matmul(
                            psE, lhsT=cW_t[:, 2 * p:2 * p + 2, :],
                            rhs=hs_t[g][:, 2 * p:2 * p + 2, o:o + w],
                            start=(p == 0), stop=(p == 3), perf_mode=DR)
                    nc.vector.tensor_scalar(
                        out=E_bf[:, s + o:s + o + w], in0=psE,
                        scalar1=1.0 / WSCALE, scalar2=bias_f,
                        op0=MUL, op1=ADD)

            def combine_front(rl, rr, tag):
                """exp + Prep + U matmuls + V mults; returns (V, psS-ready deps
                implicit in tiles)."""
                Pl = tmp.tile([L, 256], dtb, tag=f"Pl{tag}")
                Pr = tmp.tile([L, 256], dtb, tag=f"Pr{tag}")
                nc.scalar.activation(out=Pl, in_=rl, func=AF.Exp)
                nc.scalar.activation(out=Pr, in_=rr, func=AF.Exp)
                psPrep = pss.tile([128, 256], dtf, tag="psPrep")
                nc.tensor.matmul(psPrep, lhsT=rep4_t, rhs=Pl,
                                 start=True, stop=True)
                Prep = tmp.tile([128, 256], dtb, tag=f"Prep{tag}")
                nc.scalar.activation(out=Prep, in_=psPrep, func=AF.Identity)
                V = vbuf.tile([128, 8, 256], dtb, tag=f"V{tag}")
                for h in range(4):
                    psU = psu.tile([128, 2, 256], dtf, tag="psU")
                    for q in range(2):
                        c = 2 * h + q
                        nc.tensor.matmul(
                            psU[:, q, :],
                            lhsT=t2T_t[:, c * 128:(c + 1) * 128],
                            rhs=Pr, start=True, stop=True)
                    for q in range(2):
                        c = 2 * h + q
                        nc.vector.tensor_tensor(
                            out=V[:, c, :], in0=psU[:, q, :], in1=Prep, op=MUL)
                return V

            def combine_back(V, elev, r_out, tag):
                psS = pss.tile([L, 256], dtf, tag="psS")
                for c in range(8):
                    nc.tensor.matmul(psS, lhsT=sel8_t[:, c, :],
                                     rhs=V[:, c, :],
                                     start=(c == 0), stop=(c == 7))
                lnS = tmp.tile([L, 256], dtb, tag=f"lnS{tag}")
                nc.scalar.activation(out=lnS, in_=psS, func=AF.Ln)
                nc.vector.tensor_add(r_out, lnS, elev)

            # level-1 passes chase their blocks; E of the next block fills the
            # PE stall while the V multiplies run on the DVE.
            emit_E(0)
            V0 = combine_front(E_bf[:, 0:256], E_bf[:, 256:512], 0)
            emit_E(1)
            combine_back(V0, E_bf[:, 512:768], resid1[:, 0:256], 0)
            V1 = combine_front(E_bf[:, 768:1024], E_bf[:, 1024:1280], 1)
            emit_E(2)
            combine_back(V1, E_bf[:, 1280:1536], resid1[:, 256:512], 1)

            # level 2: 512 -> 256
            V2 = combine_front(resid1[:, 0:256], resid1[:, 256:512], 2)
            emit_E(3)
            combine_back(V2, E_bf[:, 1536:1792], resid2, 2)

            # outputs on the scalar HWDGE ring (input ring keeps its FIFO)
            nc.scalar.dma_start(out=outE[:, :], in_=E_bf[:, 1792:2048])
            nc.scalar.dma_start(out=outResid[:, :], in_=resid2)

    # Pin Exp/Ln/Identity to the one table set containing all three, so the
    # ACT engine loads its function table exactly once (the default picker
    # chooses per-function sets and reloads ~2.7us on every Exp<->Ln switch).
    import concourse.bacc as _bacc_mod
    from concourse.hw_specs import get_activation_tables as _gat
    _keep = "natural_log_exp_and_others"
    _pin = {AF.Exp, AF.Ln, AF.Identity, AF.Copy}

    def _gat_pinned(arch):
        t = _gat(arch)
        return {name: (funcs if name == _keep else (set(funcs) - _pin))
                for name, funcs in t.items()}

    _orig_gat = _bacc_mod.get_activation_tables
    _bacc_mod.get_activation_tables = _gat_pinned
    try:
        nc.compile()
    finally:
        _bacc_mod.get_activation_tables = _orig_gat
    _NC = nc
    return nc


def _patch_light_tail():
    """Use sem-only end-of-kernel barriers (the default drain + two full
    all-engine barriers cost ~9us of kernel tail)."""
    from concourse import tile as _tile_mod
    from concourse.vector_clock import ScopedClock

    def _dab_light(self, tick_clock, wait_clock):
        drain_inst = self.nc.sync.drain()
        wait_clock.add_sem_waits(
            drain_inst.ins, ScopedClock({None: tick_clock.global_clock})
        )
        self.nc.all_engine_barrier(sem_only=True)
        popped = self.nc._tile_sem_poison_stack.pop()
        assert popped is self._sem_poison
        self.nc.clear_and_free_semaphores(list(self.sems.allocated().values()))
        self.nc.all_engine_barrier(sem_only=True)

    _tile_mod.TileContext._drain_and_barrier = _dab_light


_patch_light_tail()


def _prep_in_maps(hidden, W, b, trans):
    """Build per-core input dicts (host-side shard/transpose/cast)."""
    W32 = (W.astype(np.float32) * WSCALE).astype(FP8)
    cW = np.ascontiguousarray(
        W32.T.reshape(8, 128, L).transpose(1, 0, 2).reshape(128, 8 * L))

    T2 = np.exp(trans.astype(np.float64)).astype(np.float32)  # [k, l, r]
    t2T = np.ascontiguousarray(T2.reshape(L * L, L).T).astype(BF16)  # [r,(k l)]

    rep4 = np.zeros((L, 128), dtype=BF16)
    for m in range(128):
        rep4[m % L, m] = BF16(1.0)
    sel8 = np.zeros((128, 8, L), dtype=BF16)
    for p in range(128):
        for c in range(8):
            sel8[p, c, 4 * c + p // 32] = BF16(1.0)

    c32 = np.zeros((L, 1153), dtype=BF16)
    c32[:, 0:1024] = t2T
    c32[:, 1024:1152] = rep4
    c32[:, 1152] = b.astype(BF16)
    c128 = np.ascontiguousarray(sel8.reshape(128, 256))

    in_maps = []
    for c in range(N_CORES):
        idx_old = _core_col_heap_index(c)               # old col -> heap row
        rows = np.zeros((COLS, INPUT_SIZE), dtype=FP8)
        real = NEWCOL_TO_OLD < 2047
        rows[real] = hidden[idx_old[NEWCOL_TO_OLD[real]]].astype(FP8)
        m = {"cW": cW, "c128": c128, "c32": c32}
        for g in range(4):
            s = int(BLOCK_STARTS[g])
            n = BLOCK_SIZES[g]
            blk = rows[s:s + n]                         # [n, 1024]
            # hsB[p, c*n + j] = blk[j, c*128 + p]
            m[f"hsB{g}"] = np.ascontiguousarray(
                blk.reshape(n, 8, 128).transpose(2, 1, 0).reshape(128, 8 * n))
        in_maps.append(m)
    return in_maps


def _host_finish(results, hidden, W, b, trans):
    """Finish levels 3..10 per core + big-tree top 3 levels, in float64."""
    Texp = np.exp(trans.astype(np.float64)).reshape(L, L * L)   # [k, (l r)]

    score = np.zeros((N_CORES, 256, L))
    elev_nat = {}   # (core, lev) -> [m, L] natural-order E
    q8 = _bitrev(np.arange(256), 8)
    for c in range(N_CORES):
        r = results[c]
        Etail = r["outE"].astype(np.float64)            # [L, 256] old 1792..2047
        resid2 = r["outResid"].astype(np.float64)       # [L, 256]
        score[c] = resid2[:, q8].T                      # node j at col bitrev(j)
        for lev in range(3, SUB_LEVELS):
            m = 1 << (10 - lev)
            qq = _bitrev(np.arange(m), 10 - lev)
            elev_nat[(c, lev)] = Etail[:, OFFS[lev] - 1792 + qq].T

    # subtree levels 3..10 (vectorized over cores)
    for lev in range(3, SUB_LEVELS):
        left = score[:, 0::2]
        right = score[:, 1::2]
        Elev = np.stack([elev_nat[(c, lev)] for c in range(N_CORES)])
        ml = left.max(axis=2, keepdims=True)
        mr = right.max(axis=2, keepdims=True)
        P = (np.exp(left - ml)[..., :, None] *
             np.exp(right - mr)[..., None, :]).reshape(N_CORES, -1, L * L)
        score = Elev + np.log(P @ Texp.T) + ml + mr

    # big-tree top: level-3 scores are the 8 subtree roots, heap nodes 7..14
    score = score.reshape(8, L)
    Etop = (hidden[:7].astype(np.float64) @ W.astype(np.float64).T
            + b.astype(np.float64))
    for d in (2, 1, 0):
        left = score[0::2]
        right = score[1::2]
        Elev = Etop[(1 << d) - 1: (1 << (d + 1)) - 1]
        ml = left.max(axis=1, keepdims=True)
        mr = right.max(axis=1, keepdims=True)
        P = (np.exp(left - ml)[:, :, None] *
             np.exp(right - mr)[:, None, :]).reshape(-1, L * L)
        score = Elev + np.log(P @ Texp.T) + ml + mr
    return score[0].astype(np.float32)


def _run_spmd(in_maps, trace=False):
    from concourse.bass_utils import run_bass_kernel_spmd
    nc = _build_bass()
    return run_bass_kernel_spmd(nc, in_maps, list(range(N_CORES)), trace=trace)


def kernel(hidden, W, b, trans):
    hidden = np.asarray(hidden, dtype=np.float32)
    W = np.asarray(W, dtype=np.float32)
    b = np.asarray(b, dtype=np.float32)
    trans = np.asarray(trans, dtype=np.float32)
    in_maps = _prep_in_maps(hidden, W, b, trans)
    res = _run_spmd(in_maps, trace=False)
    return _host_finish(res.results, hidden, W, b, trans)
